# revision 20
# baseline (speedup 1.0000x reference)
"""Trainium2 Bass kernel for nn_DeepFeatureLoss (pairwise softmax-correspondence loss).

Math (per batch b):
    P = softmax_j(-||x_i - x_j||^2),   x = points / SIGMA
    F = softmax_j(-||f1_i - f2_j||^2)
    out[b] = sum_i w_i * sum_j (P_ij - F_ij)^2

Strategy: shard rows i across 8 cores (512 rows each). Host precomputes
transposed/augmented matmul operands so the device kernel is pure compute:
    score_spatial[i,j] = (2/s^2) x_i.x_j - (1/s^2)|x_j|^2   (K=4 matmul, ones row)
    exp with per-row bias -(1/s^2)|x_i|^2  ->  exp(-||xi-xj||^2/s^2)  (<= 1, no overflow)
and similarly for features (K=33). Row sums come free via activation accum.
    sum_j (P-F)^2 = (1/s1^2) * sum_j (c*e2 - e1)^2,  c = s1/s2
computed with one fused scalar_tensor_tensor pass + split square-reduce
(ScalarE on the first 1024 cols, VectorE tensor_tensor_reduce on the rest).
Per-core partial losses [128 lanes, B] are summed on host.
"""

import os
import sys

import numpy as np

sys.path.insert(0, "/opt/trn_rl_repo")

import concourse.bass as bass
import concourse.tile as tile
from concourse import mybir
from concourse.bass_utils import run_bass_kernel_spmd

SIGMA = 0.05
B = 2
N = 4096
D = 32
NCORES = 8
RPC = N // NCORES          # rows per core = 512
TILES = RPC // 128         # i-tiles per core per batch = 4
KF = D + 1                 # feature contraction with ones row
KS = 4                     # spatial contraction (3 coords + ones row)
ACT_COLS = 1024            # columns of the square-reduce done on ScalarE

FP = mybir.dt.float32
FPR = mybir.dt.float32r    # 4x faster PE streaming, fp32 data
AX = mybir.AxisListType
OP = mybir.AluOpType
AF = mybir.ActivationFunctionType

LAST_RESULT = None         # test harness introspection


def _fix_walrus_incompat(nc):
    """This container's walrus codegen fits exactly ONE sync-wait per engine
    instruction struct (Tile's scheduler freely emits several) and rejects the
    EVENT_SEMAPHORE_RANGE_CLEAR raw-ISA instruction Tile emits at context
    exit. Rewrite: (a) every multi-wait instruction becomes (n-1) same-engine
    EventSemaphore waits followed by the instruction with the final wait;
    (b) the range-clear becomes one sem-wr-imm(0) EventSemaphore per sem."""
    import re

    from bass_rust import SyncInfo, SyncUpdate

    fn = nc.m.functions[0]
    originals = [(blk, list(blk.instructions)) for blk in fn.blocks]
    rebuilt = []
    for blk, insts in originals:
        out = []
        for inst in insts:
            tname = type(inst).__name__
            si = inst.sync_info
            if tname == "InstISA" and "EVENT_SEMAPHORE_RANGE_CLEAR" in inst.concise():
                m = re.search(r"range_first=(\d+) range_last=(\d+)", inst.concise())
                first, last = int(m.group(1)), int(m.group(2))
                for sem in range(first, last + 1):
                    ev = mybir.InstEventSemaphore(
                        name=nc.get_next_instruction_name(),
                        engine=inst.engine,
                        sync_info=SyncInfo(
                            on_wait=list(si.on_wait) if si and sem == first else [],
                            on_update=[
                                SyncUpdate(
                                    sync_type="semaphore",
                                    id=sem,
                                    ant_name=f"semclear_{sem}",
                                    update_mode="sem-wr-imm",
                                    update_value=0,
                                    update_reg=None,
                                )
                            ],
                        ),
                    )
                    nc.register_instruction(ev, overwrite=True)
                    out.append(ev)
                continue
            if si is not None and len(si.on_wait) > 1:
                waits = list(si.on_wait)
                for w in waits[:-1]:
                    ev = mybir.InstEventSemaphore(
                        name=nc.get_next_instruction_name(),
                        engine=inst.engine,
                        sync_info=SyncInfo(on_wait=[w], on_update=[]),
                    )
                    nc.register_instruction(ev, overwrite=True)
                    out.append(ev)
                inst.sync_info = SyncInfo(
                    on_wait=[waits[-1]], on_update=list(si.on_update)
                )
            out.append(inst)
        rebuilt.append((blk, out))
    for blk, out in rebuilt:
        blk.instructions[:] = out


def _build_nc(spat_dtype, feat_dtype):
    nc = bass.Bass()

    # rhs (cols 0:N) and the local lhsT block (cols N:N+RPC) share one DRAM
    # tensor so each matmul family depends on exactly ONE input DMA — walrus's
    # core_v3 LDWEIGHTS struct only fits a single sync-wait.
    spat_comb = nc.dram_tensor("spat_comb", [B, KS, N + RPC], spat_dtype, kind="ExternalInput")
    feat_comb = nc.dram_tensor("feat_comb", [B, KF, N + RPC], feat_dtype, kind="ExternalInput")
    # biases + weights packed partition-major: smalls[p, tensor*2*TILES + b*TILES + t]
    # = value for row t*128+p of batch b. One contiguous 96B-per-partition DMA
    # instead of three 4B-scatter DMAs (which cost ~1 packet per element).
    smalls = nc.dram_tensor("smalls", [128, 3 * B * TILES], FP, kind="ExternalInput")
    out = nc.dram_tensor("out", [B, 128], FP, kind="ExternalOutput")

    with tile.TileContext(nc) as tc:
        with (
            tc.tile_pool(name="const", bufs=1) as cpool,
            tc.tile_pool(name="psum", bufs=2, space="PSUM") as ppool,
            tc.tile_pool(name="ebuf", bufs=2) as epool,
            tc.tile_pool(name="junk", bufs=2) as jpool,
            tc.tile_pool(name="small", bufs=3) as spool,
            tc.tile_pool(name="accs", bufs=1) as apool,
        ):
            # --- load constants ---
            sm = cpool.tile([128, 3 * B * TILES], FP, tag="smalls")
            nc.sync.dma_start(sm[:], smalls[:])
            bx = [sm[:, b * TILES : (b + 1) * TILES] for b in range(B)]
            bf = [sm[:, (B + b) * TILES : (B + b + 1) * TILES] for b in range(B)]
            wt = [sm[:, (2 * B + b) * TILES : (2 * B + b + 1) * TILES] for b in range(B)]
            scomb, fcomb = [], []
            for b in range(B):
                t_ = cpool.tile([KS, N + RPC], spat_dtype, tag=f"scomb{b}")
                nc.sync.dma_start(t_[:], spat_comb[b])
                scomb.append(t_)
                t_ = cpool.tile([KF, N + RPC], feat_dtype, tag=f"fcomb{b}")
                nc.sync.dma_start(t_[:], feat_comb[b])
                fcomb.append(t_)

            outsb = apool.tile([128, B], FP, tag="outsb")

            # PE p-state warmup: ~24 dense bf16 matmuls ramp the tensor engine
            # to full clock before the fp32 spatial matmuls start (fp32 at mid
            # p-state costs 2x; see P3 "HAM warmup").
            n_warm = int(os.environ.get("DFL_WARMUP", "24"))
            if n_warm:
                wsrc = cpool.tile([128, 512], mybir.dt.bfloat16, tag="warm")
                nc.gpsimd.memset(wsrc[:], 1.0)
                for k in range(n_warm):
                    pw = ppool.tile([128, 2048], FP, tag="ps")
                    nc.tensor.matmul(
                        pw[:, 0:512], wsrc[:, 0:128], wsrc[:], start=True, stop=True
                    )

            for b in range(B):
                accq = apool.tile([128, TILES], FP, tag=f"accq{b}")
                for t in range(TILES):
                    spart = spool.tile([128, 4], FP, tag="spart")
                    e1 = epool.tile([128, N], FP, tag="e1")
                    e2 = epool.tile([128, N], FP, tag="e2")
                    # spatial & feature score blocks + exp, 2048 cols at a time
                    for half, (comb_t, bias_t, ebuf) in enumerate(
                        (
                            (scomb[b], bx[b], e1),
                            (scomb[b], bx[b], e1),
                            (fcomb[b], bf[b], e2),
                            (fcomb[b], bf[b], e2),
                        )
                    ):
                        half_is = half % 2
                        col0 = half_is * 2048
                        ps = ppool.tile([128, 2048], FP, tag="ps")
                        for k in range(4):
                            nc.tensor.matmul(
                                ps[:, k * 512 : (k + 1) * 512],
                                comb_t[:, N + t * 128 : N + (t + 1) * 128],
                                comb_t[:, col0 + k * 512 : col0 + (k + 1) * 512],
                                start=True,
                                stop=True,
                            )
                        nc.scalar.activation(
                            ebuf[:, col0 : col0 + 2048],
                            ps[:],
                            AF.Exp,
                            bias=bias_t[:, t : t + 1],
                            accum_out=spart[:, 2 * (half // 2) + half_is : 2 * (half // 2) + half_is + 1],
                        )
                    # row sums s1 (spatial), s2 (feature) and derived scalars
                    sums = spool.tile([128, 2], FP, tag="sums")
                    nc.vector.tensor_reduce(
                        sums[:], spart[:].rearrange("p (m c) -> p m c", c=2), axis=AX.X, op=OP.add
                    )
                    rec = spool.tile([128, 2], FP, tag="rec")
                    nc.vector.reciprocal(rec[:], sums[:])
                    cc = spool.tile([128, 1], FP, tag="cc")
                    nc.vector.tensor_tensor(cc[:], sums[:, 0:1], rec[:, 1:2], op=OP.mult)
                    # d = c*e2 - e1 (in place into e2); then sum d^2 split S/V
                    nc.vector.scalar_tensor_tensor(
                        e2[:], e2[:], cc[:], e1[:], op0=OP.mult, op1=OP.subtract
                    )
                    qa = spool.tile([128, 1], FP, tag="qa")
                    qv = spool.tile([128, 1], FP, tag="qv")
                    junk = jpool.tile([128, ACT_COLS], FP, tag="junk")
                    nc.scalar.activation(
                        junk[:],
                        e2[:, 0:ACT_COLS],
                        AF.Square,
                        scale=rec[:, 0:1],
                        accum_out=qa[:],
                    )
                    nc.vector.scalar_tensor_tensor(
                        out=e2[:, ACT_COLS:],
                        in0=e2[:, ACT_COLS:],
                        scalar=1.0,
                        in1=e2[:, ACT_COLS:],
                        op0=OP.mult,
                        op1=OP.mult,
                        accum_out=qv[:],
                    )
                    # loss rows = qa + inv1^2 * qv
                    i2 = spool.tile([128, 1], FP, tag="i2")
                    nc.vector.tensor_tensor(i2[:], rec[:, 0:1], rec[:, 0:1], op=OP.mult)
                    qv2 = spool.tile([128, 1], FP, tag="qv2")
                    nc.vector.tensor_tensor(qv2[:], qv[:], i2[:], op=OP.mult)
                    nc.vector.tensor_tensor(accq[:, t : t + 1], qa[:], qv2[:], op=OP.add)
                # weighted reduce over this batch's 4 i-tiles
                lw = spool.tile([128, TILES], FP, tag="lw")
                nc.vector.tensor_tensor(lw[:], accq[:], wt[b], op=OP.mult)
                nc.vector.tensor_reduce(outsb[:, b : b + 1], lw[:], axis=AX.X, op=OP.add)

            for b in range(B):
                nc.sync.dma_start(out[b].rearrange("(p o) -> p o", o=1), outsb[:, b : b + 1])

    _fix_walrus_incompat(nc)
    return nc


_NC_CACHE = {}


def _get_nc(use_fp32r=True):
    """Default: fp32 spatial scores (magnitudes ~1e3 make fp32r's relaxed
    precision visible in the loss), fp32r feature scores (magnitudes ~1e2,
    error negligible). DFL_NO_FP32R=1 forces fp32 everywhere;
    DFL_ALL_FP32R=1 forces fp32r everywhere."""
    if os.environ.get("DFL_ALL_FP32R", "") == "1":
        key, dts = "rr", (FPR, FPR)
    elif not use_fp32r:
        key, dts = "ff", (FP, FP)
    else:
        key, dts = "fr", (FP, FPR)
    if key not in _NC_CACHE:
        _NC_CACHE[key] = _build_nc(*dts)
    return _NC_CACHE[key]


def _prep_inputs(points, pointfea1, pointfea2, weights):
    """Host-side sharding + operand layout. Returns per-core input maps."""
    s2inv = np.float64(1.0) / (SIGMA * SIGMA)
    x = points.astype(np.float64)        # [B, N, 3]
    f1 = pointfea1.astype(np.float64)    # [B, N, D]
    f2 = pointfea2.astype(np.float64)
    w = weights.astype(np.float32)

    xT = np.swapaxes(x, 1, 2)            # [B, 3, N]
    f1T = np.swapaxes(f1, 1, 2)          # [B, D, N]
    f2T = np.swapaxes(f2, 1, 2)

    xn = np.sum(x * x, axis=2)           # [B, N]
    f1n = np.sum(f1 * f1, axis=2)
    f2n = np.sum(f2 * f2, axis=2)

    in_maps = []
    for c in range(NCORES):
        sl = slice(c * RPC, (c + 1) * RPC)
        spat_comb = np.empty((B, KS, N + RPC), np.float32)
        spat_comb[:, :3, :N] = 2.0 * s2inv * xT
        spat_comb[:, 3, :N] = -s2inv * xn
        spat_comb[:, :3, N:] = xT[:, :, sl]
        spat_comb[:, 3, N:] = 1.0
        feat_comb = np.empty((B, KF, N + RPC), np.float32)
        feat_comb[:, :D, :N] = 2.0 * f2T
        feat_comb[:, D, :N] = -f2n
        feat_comb[:, :D, N:] = f1T[:, :, sl]
        feat_comb[:, D, N:] = 1.0
        # smalls[p, tensor*B*TILES + b*TILES + t] = value for row t*128+p
        smalls = np.empty((128, 3 * B * TILES), np.float32)
        for b in range(B):
            bx = (-s2inv * xn[b, sl]).astype(np.float32).reshape(TILES, 128)
            bfv = (-f1n[b, sl]).astype(np.float32).reshape(TILES, 128)
            wv = w[b, sl].reshape(TILES, 128)
            smalls[:, b * TILES : (b + 1) * TILES] = bx.T
            smalls[:, (B + b) * TILES : (B + b + 1) * TILES] = bfv.T
            smalls[:, (2 * B + b) * TILES : (2 * B + b + 1) * TILES] = wv.T
        in_maps.append(
            {
                "spat_comb": spat_comb,
                "feat_comb": feat_comb,
                "smalls": smalls,
            }
        )
    return in_maps


def kernel(points, pointfea1, pointfea2, weights):
    global LAST_RESULT
    in_maps = _prep_inputs(points, pointfea1, pointfea2, weights)
    nc = _get_nc(use_fp32r=os.environ.get("DFL_NO_FP32R", "") != "1")
    res = run_bass_kernel_spmd(nc, in_maps, core_ids=list(range(NCORES)))
    LAST_RESULT = res
    total = np.zeros(B, np.float64)
    for m in res.results:
        total += m["out"].astype(np.float64).sum(axis=1)
    return total.astype(np.float32)


# revision 28
# speedup vs baseline: 1.1745x; 1.1745x over previous
"""Trainium2 Bass kernel for nn_DeepFeatureLoss (pairwise softmax-correspondence loss).

Math (per batch b):
    P = softmax_j(-||x_i - x_j||^2),   x = points / SIGMA
    F = softmax_j(-||f1_i - f2_j||^2)
    out[b] = sum_i w_i * sum_j (P_ij - F_ij)^2

Strategy: shard rows i across 8 cores (512 rows each). Host precomputes
transposed/augmented matmul operands so the device kernel is pure compute:
    score_spatial[i,j] = (2/s^2) x_i.x_j - (1/s^2)|x_j|^2   (K=4 matmul, ones row)
    exp with per-row bias -(1/s^2)|x_i|^2  ->  exp(-||xi-xj||^2/s^2)  (<= 1, no overflow)
and similarly for features (K=33). Row sums come free via activation accum.
    sum_j (P-F)^2 = (1/s1^2) * sum_j (c*e2 - e1)^2,  c = s1/s2
computed with one fused scalar_tensor_tensor pass + split square-reduce
(ScalarE on the first 1024 cols, VectorE tensor_tensor_reduce on the rest).
Per-core partial losses [128 lanes, B] are summed on host.
"""

import os
import sys

import numpy as np

sys.path.insert(0, "/opt/trn_rl_repo")

import concourse.bass as bass
import concourse.tile as tile
from concourse import mybir
from concourse.bass_utils import run_bass_kernel_spmd

SIGMA = 0.05
B = 2
N = 4096
D = 32
NCORES = 8
RPC = N // NCORES          # rows per core = 512
TILES = RPC // 128         # i-tiles per core per batch = 4
KF = D + 1                 # feature contraction with ones row
KS = 4                     # spatial contraction (3 coords + ones row)
ACT_COLS = 1024            # columns of the square-reduce done on ScalarE

FP = mybir.dt.float32
FPR = mybir.dt.float32r    # 4x faster PE streaming, fp32 data
AX = mybir.AxisListType
OP = mybir.AluOpType
AF = mybir.ActivationFunctionType

LAST_RESULT = None         # test harness introspection


def _fix_walrus_incompat(nc):
    """This container's walrus codegen fits exactly ONE sync-wait per engine
    instruction struct (Tile's scheduler freely emits several) and rejects the
    EVENT_SEMAPHORE_RANGE_CLEAR raw-ISA instruction Tile emits at context
    exit. Rewrite: (a) every multi-wait instruction becomes (n-1) same-engine
    EventSemaphore waits followed by the instruction with the final wait;
    (b) the range-clear becomes one sem-wr-imm(0) EventSemaphore per sem."""
    import re

    from bass_rust import SyncInfo, SyncUpdate

    fn = nc.m.functions[0]
    originals = [(blk, list(blk.instructions)) for blk in fn.blocks]
    rebuilt = []
    for blk, insts in originals:
        out = []
        for inst in insts:
            tname = type(inst).__name__
            si = inst.sync_info
            if tname == "InstISA" and "EVENT_SEMAPHORE_RANGE_CLEAR" in inst.concise():
                m = re.search(r"range_first=(\d+) range_last=(\d+)", inst.concise())
                first, last = int(m.group(1)), int(m.group(2))
                for sem in range(first, last + 1):
                    ev = mybir.InstEventSemaphore(
                        name=nc.get_next_instruction_name(),
                        engine=inst.engine,
                        sync_info=SyncInfo(
                            on_wait=list(si.on_wait) if si and sem == first else [],
                            on_update=[
                                SyncUpdate(
                                    sync_type="semaphore",
                                    id=sem,
                                    ant_name=f"semclear_{sem}",
                                    update_mode="sem-wr-imm",
                                    update_value=0,
                                    update_reg=None,
                                )
                            ],
                        ),
                    )
                    nc.register_instruction(ev, overwrite=True)
                    out.append(ev)
                continue
            if si is not None and len(si.on_wait) > 1:
                waits = list(si.on_wait)
                for w in waits[:-1]:
                    ev = mybir.InstEventSemaphore(
                        name=nc.get_next_instruction_name(),
                        engine=inst.engine,
                        sync_info=SyncInfo(on_wait=[w], on_update=[]),
                    )
                    nc.register_instruction(ev, overwrite=True)
                    out.append(ev)
                inst.sync_info = SyncInfo(
                    on_wait=[waits[-1]], on_update=list(si.on_update)
                )
            out.append(inst)
        rebuilt.append((blk, out))
    for blk, out in rebuilt:
        blk.instructions[:] = out


def _build_nc(spat_dtype, feat_dtype, spat_split=False):
    nc = bass.Bass()

    # rhs (cols 0:N) and the local lhsT block (cols N:N+RPC) share one DRAM
    # tensor so each matmul family depends on exactly ONE input DMA — walrus's
    # core_v3 LDWEIGHTS struct only fits a single sync-wait.
    # spat_split: fp16 hi/lo operand pair, spatial scores = hi.hi + hi.lo +
    # lo.hi accumulated in PSUM (22-bit effective mantissa at full PE rate;
    # fp32 matmul runs at quarter rate and caps the PE clock). K=5: rows 3/4
    # carry the column-norm term split into nh+nl+n2 across the products.
    ksp = 5 if spat_split else KS
    if spat_split:
        spat_hi = nc.dram_tensor("spat_hi", [B, ksp, N + RPC], spat_dtype, kind="ExternalInput")
        spat_lo = nc.dram_tensor("spat_lo", [B, ksp, N + RPC], spat_dtype, kind="ExternalInput")
    else:
        spat_comb = nc.dram_tensor("spat_comb", [B, ksp, N + RPC], spat_dtype, kind="ExternalInput")
    feat_comb = nc.dram_tensor("feat_comb", [B, KF, N + RPC], feat_dtype, kind="ExternalInput")
    # biases + weights packed partition-major: smalls[p, tensor*2*TILES + b*TILES + t]
    # = value for row t*128+p of batch b. One contiguous 96B-per-partition DMA
    # instead of three 4B-scatter DMAs (which cost ~1 packet per element).
    smalls = nc.dram_tensor("smalls", [128, 3 * B * TILES], FP, kind="ExternalInput")
    out = nc.dram_tensor("out", [B, 128], FP, kind="ExternalOutput")

    with tile.TileContext(nc) as tc:
        with (
            tc.tile_pool(name="const", bufs=1) as cpool,
            tc.tile_pool(name="psum", bufs=2, space="PSUM") as ppool,
            tc.tile_pool(name="ebuf", bufs=2) as epool,
            tc.tile_pool(name="junk", bufs=2) as jpool,
            tc.tile_pool(name="small", bufs=3) as spool,
            tc.tile_pool(name="accs", bufs=1) as apool,
        ):
            # --- load constants ---
            sm = cpool.tile([128, 3 * B * TILES], FP, tag="smalls")
            nc.sync.dma_start(sm[:], smalls[:])
            bx = [sm[:, b * TILES : (b + 1) * TILES] for b in range(B)]
            bf = [sm[:, (B + b) * TILES : (B + b + 1) * TILES] for b in range(B)]
            wt = [sm[:, (2 * B + b) * TILES : (2 * B + b + 1) * TILES] for b in range(B)]
            def load_split(dram, b, shape, dt_, tag):
                # column-chunked DMA: 4 queues in parallel instead of one
                # serialized ~30us transfer
                t_ = cpool.tile(shape, dt_, tag=tag)
                cw = (N + RPC) // 4
                for c in range(4):
                    nc.sync.dma_start(
                        t_[:, c * cw : (c + 1) * cw], dram[b][:, c * cw : (c + 1) * cw]
                    )
                return t_

            scomb, slo_t, fcomb = [], [], []
            for b in range(B):
                if spat_split:
                    scomb.append(load_split(spat_hi, b, [ksp, N + RPC], spat_dtype, f"shi{b}"))
                    slo_t.append(load_split(spat_lo, b, [ksp, N + RPC], spat_dtype, f"slo{b}"))
                else:
                    scomb.append(load_split(spat_comb, b, [ksp, N + RPC], spat_dtype, f"shi{b}"))
                    slo_t.append(None)
                fcomb.append(load_split(feat_comb, b, [KF, N + RPC], feat_dtype, f"fcomb{b}"))

            outsb = apool.tile([128, B], FP, tag="outsb")

            # PE p-state warmup: ~24 dense bf16 matmuls ramp the tensor engine
            # to full clock before the fp32 spatial matmuls start (fp32 at mid
            # p-state costs 2x; see P3 "HAM warmup").
            n_warm = int(os.environ.get("DFL_WARMUP", "24"))
            if n_warm:
                wsrc = cpool.tile([128, 512], mybir.dt.bfloat16, tag="warm")
                nc.gpsimd.memset(wsrc[:], 1.0)
                for k in range(n_warm):
                    pw = ppool.tile([128, 2048], FP, tag="ps")
                    nc.tensor.matmul(
                        pw[:, 0:512], wsrc[:, 0:128], wsrc[:], start=True, stop=True
                    )

            for b in range(B):
                accq = apool.tile([128, TILES], FP, tag=f"accq{b}")
                for t in range(TILES):
                    spart = spool.tile([128, 4], FP, tag="spart")
                    e1 = epool.tile([128, N], FP, tag="e1")
                    e2 = epool.tile([128, N], FP, tag="e2")
                    # spatial & feature score blocks + exp, 2048 cols at a time
                    for half, (comb_t, lo_t, bias_t, ebuf) in enumerate(
                        (
                            (scomb[b], slo_t[b], bx[b], e1),
                            (scomb[b], slo_t[b], bx[b], e1),
                            (fcomb[b], None, bf[b], e2),
                            (fcomb[b], None, bf[b], e2),
                        )
                    ):
                        half_is = half % 2
                        col0 = half_is * 2048
                        ps = ppool.tile([128, 2048], FP, tag="ps")
                        lhs_sl = slice(N + t * 128, N + (t + 1) * 128)
                        for k in range(4):
                            rhs_sl = slice(col0 + k * 512, col0 + (k + 1) * 512)
                            if lo_t is None:
                                nc.tensor.matmul(
                                    ps[:, k * 512 : (k + 1) * 512],
                                    comb_t[:, lhs_sl],
                                    comb_t[:, rhs_sl],
                                    start=True,
                                    stop=True,
                                )
                            else:
                                # hi.hi + hi.lo + lo.hi accumulate in PSUM
                                for mi, (lt, rt) in enumerate(
                                    ((comb_t, comb_t), (comb_t, lo_t), (lo_t, comb_t))
                                ):
                                    nc.tensor.matmul(
                                        ps[:, k * 512 : (k + 1) * 512],
                                        lt[:, lhs_sl],
                                        rt[:, rhs_sl],
                                        start=(mi == 0),
                                        stop=(mi == 2),
                                    )
                        nc.scalar.activation(
                            ebuf[:, col0 : col0 + 2048],
                            ps[:],
                            AF.Exp,
                            bias=bias_t[:, t : t + 1],
                            accum_out=spart[:, 2 * (half // 2) + half_is : 2 * (half // 2) + half_is + 1],
                        )
                    # row sums s1 (spatial), s2 (feature) and derived scalars
                    sums = spool.tile([128, 2], FP, tag="sums")
                    nc.vector.tensor_reduce(
                        sums[:], spart[:].rearrange("p (m c) -> p m c", c=2), axis=AX.X, op=OP.add
                    )
                    rec = spool.tile([128, 2], FP, tag="rec")
                    nc.vector.reciprocal(rec[:], sums[:])
                    cc = spool.tile([128, 1], FP, tag="cc")
                    nc.vector.tensor_tensor(cc[:], sums[:, 0:1], rec[:, 1:2], op=OP.mult)
                    # d = c*e2 - e1 (in place into e2); then sum d^2 split S/V
                    nc.vector.scalar_tensor_tensor(
                        e2[:], e2[:], cc[:], e1[:], op0=OP.mult, op1=OP.subtract
                    )
                    qa = spool.tile([128, 1], FP, tag="qa")
                    qv = spool.tile([128, 1], FP, tag="qv")
                    junk = jpool.tile([128, ACT_COLS], FP, tag="junk")
                    nc.scalar.activation(
                        junk[:],
                        e2[:, 0:ACT_COLS],
                        AF.Square,
                        scale=rec[:, 0:1],
                        accum_out=qa[:],
                    )
                    nc.vector.scalar_tensor_tensor(
                        out=e2[:, ACT_COLS:],
                        in0=e2[:, ACT_COLS:],
                        scalar=1.0,
                        in1=e2[:, ACT_COLS:],
                        op0=OP.mult,
                        op1=OP.mult,
                        accum_out=qv[:],
                    )
                    # loss rows = qa + inv1^2 * qv
                    i2 = spool.tile([128, 1], FP, tag="i2")
                    nc.vector.tensor_tensor(i2[:], rec[:, 0:1], rec[:, 0:1], op=OP.mult)
                    qv2 = spool.tile([128, 1], FP, tag="qv2")
                    nc.vector.tensor_tensor(qv2[:], qv[:], i2[:], op=OP.mult)
                    nc.vector.tensor_tensor(accq[:, t : t + 1], qa[:], qv2[:], op=OP.add)
                # weighted reduce over this batch's 4 i-tiles
                lw = spool.tile([128, TILES], FP, tag="lw")
                nc.vector.tensor_tensor(lw[:], accq[:], wt[b], op=OP.mult)
                nc.vector.tensor_reduce(outsb[:, b : b + 1], lw[:], axis=AX.X, op=OP.add)

            for b in range(B):
                nc.sync.dma_start(out[b].rearrange("(p o) -> p o", o=1), outsb[:, b : b + 1])

    _fix_walrus_incompat(nc)
    return nc


_NC_CACHE = {}


def _get_nc(use_fp32r=True):
    """Default: fp32 spatial scores (magnitudes ~1e3 make fp32r's relaxed
    precision visible in the loss), fp32r feature scores (magnitudes ~1e2,
    error negligible). DFL_NO_FP32R=1 forces fp32 everywhere;
    DFL_ALL_FP32R=1 forces fp32r everywhere."""
    if os.environ.get("DFL_ALL_FP32R", "") == "1":
        key, dts, split = "rr", (FPR, FPR), False
    elif os.environ.get("DFL_SPAT_FP32", "") == "1":
        key, dts, split = "fr", (FP, FPR), False
    elif not use_fp32r:
        key, dts, split = "ff", (FP, FP), False
    else:
        key, dts, split = "h3", (mybir.dt.float16, FPR), True
    if key not in _NC_CACHE:
        _NC_CACHE[key] = (_build_nc(*dts, spat_split=split), split)
    return _NC_CACHE[key]


def _prep_inputs(points, pointfea1, pointfea2, weights, spat_split=True):
    """Host-side sharding + operand layout. Returns per-core input maps."""
    s2inv = np.float64(1.0) / (SIGMA * SIGMA)
    x = points.astype(np.float64)        # [B, N, 3]
    f1 = pointfea1.astype(np.float64)    # [B, N, D]
    f2 = pointfea2.astype(np.float64)
    w = weights.astype(np.float32)

    xT = np.swapaxes(x, 1, 2)            # [B, 3, N]
    f1T = np.swapaxes(f1, 1, 2)          # [B, D, N]
    f2T = np.swapaxes(f2, 1, 2)

    xn = np.sum(x * x, axis=2)           # [B, N]
    f1n = np.sum(f1 * f1, axis=2)
    f2n = np.sum(f2 * f2, axis=2)

    in_maps = []
    for c in range(NCORES):
        sl = slice(c * RPC, (c + 1) * RPC)
        if spat_split:
            # fp16 hi/lo pair, K=5. rhs cols: rows0-2 = y=(2/s^2)x_j split,
            # rows3/4 = column norm n=-(1/s^2)|x_j|^2 split into nh+nl+n2
            # spread so products hi.hi/hi.lo/lo.hi pick up each piece once.
            # lhsT cols: rows0-2 = x_i split, hi row3=1 row4=0, lo row3=0 row4=1.
            y = 2.0 * s2inv * xT
            n = -s2inv * xn
            yh = y.astype(np.float16)
            yl = (y - yh.astype(np.float64)).astype(np.float16)
            nh = n.astype(np.float16)
            nl = (n - nh.astype(np.float64)).astype(np.float16)
            n2 = (n - nh.astype(np.float64) - nl.astype(np.float64)).astype(np.float16)
            xh = xT[:, :, sl].astype(np.float16)
            xl = (xT[:, :, sl] - xh.astype(np.float64)).astype(np.float16)
            hi = np.zeros((B, 5, N + RPC), np.float16)
            lo = np.zeros((B, 5, N + RPC), np.float16)
            hi[:, :3, :N] = yh
            hi[:, 3, :N] = nh
            hi[:, 4, :N] = n2
            hi[:, :3, N:] = xh
            hi[:, 3, N:] = 1.0
            lo[:, :3, :N] = yl
            lo[:, 3, :N] = nl
            lo[:, :3, N:] = xl
            lo[:, 4, N:] = 1.0
        else:
            spat_comb = np.empty((B, KS, N + RPC), np.float32)
            spat_comb[:, :3, :N] = 2.0 * s2inv * xT
            spat_comb[:, 3, :N] = -s2inv * xn
            spat_comb[:, :3, N:] = xT[:, :, sl]
            spat_comb[:, 3, N:] = 1.0
        feat_comb = np.empty((B, KF, N + RPC), np.float32)
        feat_comb[:, :D, :N] = 2.0 * f2T
        feat_comb[:, D, :N] = -f2n
        feat_comb[:, :D, N:] = f1T[:, :, sl]
        feat_comb[:, D, N:] = 1.0
        # smalls[p, tensor*B*TILES + b*TILES + t] = value for row t*128+p
        smalls = np.empty((128, 3 * B * TILES), np.float32)
        for b in range(B):
            bx = (-s2inv * xn[b, sl]).astype(np.float32).reshape(TILES, 128)
            bfv = (-f1n[b, sl]).astype(np.float32).reshape(TILES, 128)
            wv = w[b, sl].reshape(TILES, 128)
            smalls[:, b * TILES : (b + 1) * TILES] = bx.T
            smalls[:, (B + b) * TILES : (B + b + 1) * TILES] = bfv.T
            smalls[:, (2 * B + b) * TILES : (2 * B + b + 1) * TILES] = wv.T
        m = {"feat_comb": feat_comb, "smalls": smalls}
        if spat_split:
            m["spat_hi"] = hi
            m["spat_lo"] = lo
        else:
            m["spat_comb"] = spat_comb
        in_maps.append(m)
    return in_maps


def kernel(points, pointfea1, pointfea2, weights):
    global LAST_RESULT
    nc, split = _get_nc(use_fp32r=os.environ.get("DFL_NO_FP32R", "") != "1")
    in_maps = _prep_inputs(points, pointfea1, pointfea2, weights, spat_split=split)
    res = run_bass_kernel_spmd(nc, in_maps, core_ids=list(range(NCORES)))
    LAST_RESULT = res
    total = np.zeros(B, np.float64)
    for m in res.results:
        total += m["out"].astype(np.float64).sum(axis=1)
    return total.astype(np.float32)


# revision 34
# speedup vs baseline: 1.6807x; 1.4309x over previous
"""Trainium2 Bass kernel for nn_DeepFeatureLoss (pairwise softmax-correspondence loss).

Math (per batch b):
    P = softmax_j(-||x_i - x_j||^2),   x = points / SIGMA
    F = softmax_j(-||f1_i - f2_j||^2)
    out[b] = sum_i w_i * sum_j (P_ij - F_ij)^2

Strategy: shard rows i across 8 cores (512 rows each). Host precomputes
transposed/augmented matmul operands so the device kernel is pure compute:
    score_spatial[i,j] = (2/s^2) x_i.x_j - (1/s^2)|x_j|^2   (K=4 matmul, ones row)
    exp with per-row bias -(1/s^2)|x_i|^2  ->  exp(-||xi-xj||^2/s^2)  (<= 1, no overflow)
and similarly for features (K=33). Row sums come free via activation accum.
    sum_j (P-F)^2 = (1/s1^2) * sum_j (c*e2 - e1)^2,  c = s1/s2
computed with one fused scalar_tensor_tensor pass + split square-reduce
(ScalarE on the first 1024 cols, VectorE tensor_tensor_reduce on the rest).
Per-core partial losses [128 lanes, B] are summed on host.
"""

import os
import sys

import numpy as np

sys.path.insert(0, "/opt/trn_rl_repo")

import concourse.bass as bass
import concourse.tile as tile
from concourse import mybir
from concourse.bass_utils import run_bass_kernel_spmd

SIGMA = 0.05
B = 2
N = 4096
D = 32
NCORES = 8
RPC = N // NCORES          # rows per core = 512
TILES = RPC // 128         # i-tiles per core per batch = 4
KF = D + 1                 # feature contraction with ones row
KS = 4                     # spatial contraction (3 coords + ones row)
ACT_COLS = 768             # columns of the square-reduce done on ScalarE

FP = mybir.dt.float32
FPR = mybir.dt.float32r    # 4x faster PE streaming, fp32 data
AX = mybir.AxisListType
OP = mybir.AluOpType
AF = mybir.ActivationFunctionType

LAST_RESULT = None         # test harness introspection


def _fix_walrus_incompat(nc):
    """This container's walrus codegen fits exactly ONE sync-wait per engine
    instruction struct (Tile's scheduler freely emits several) and rejects the
    EVENT_SEMAPHORE_RANGE_CLEAR raw-ISA instruction Tile emits at context
    exit. Rewrite: (a) every multi-wait instruction becomes (n-1) same-engine
    EventSemaphore waits followed by the instruction with the final wait;
    (b) the range-clear becomes one sem-wr-imm(0) EventSemaphore per sem."""
    import re

    from bass_rust import SyncInfo, SyncUpdate

    fn = nc.m.functions[0]
    originals = [(blk, list(blk.instructions)) for blk in fn.blocks]
    rebuilt = []
    for blk, insts in originals:
        out = []
        for inst in insts:
            tname = type(inst).__name__
            si = inst.sync_info
            if tname == "InstISA" and "EVENT_SEMAPHORE_RANGE_CLEAR" in inst.concise():
                m = re.search(r"range_first=(\d+) range_last=(\d+)", inst.concise())
                first, last = int(m.group(1)), int(m.group(2))
                for sem in range(first, last + 1):
                    ev = mybir.InstEventSemaphore(
                        name=nc.get_next_instruction_name(),
                        engine=inst.engine,
                        sync_info=SyncInfo(
                            on_wait=list(si.on_wait) if si and sem == first else [],
                            on_update=[
                                SyncUpdate(
                                    sync_type="semaphore",
                                    id=sem,
                                    ant_name=f"semclear_{sem}",
                                    update_mode="sem-wr-imm",
                                    update_value=0,
                                    update_reg=None,
                                )
                            ],
                        ),
                    )
                    nc.register_instruction(ev, overwrite=True)
                    out.append(ev)
                continue
            if si is not None and len(si.on_wait) > 1:
                waits = list(si.on_wait)
                for w in waits[:-1]:
                    ev = mybir.InstEventSemaphore(
                        name=nc.get_next_instruction_name(),
                        engine=inst.engine,
                        sync_info=SyncInfo(on_wait=[w], on_update=[]),
                    )
                    nc.register_instruction(ev, overwrite=True)
                    out.append(ev)
                inst.sync_info = SyncInfo(
                    on_wait=[waits[-1]], on_update=list(si.on_update)
                )
            out.append(inst)
        rebuilt.append((blk, out))
    for blk, out in rebuilt:
        blk.instructions[:] = out


def _build_nc(spat_dtype, feat_dtype, spat_split=False):
    nc = bass.Bass()

    # rhs (cols 0:N) and the local lhsT block (cols N:N+RPC) share one DRAM
    # tensor so each matmul family depends on exactly ONE input DMA — walrus's
    # core_v3 LDWEIGHTS struct only fits a single sync-wait.
    # spat_split: fp16 hi/lo decomposition of the spatial operands; the three
    # products hi.hi + hi.lo + lo.hi (22-bit effective mantissa) are STACKED
    # along the contraction axis — lhsT rows [hi;hi;lo], rhs rows [hi;lo;hi],
    # K=15 — so the whole compensated product is ONE full-rate fp16 matmul
    # (fp32 matmul runs at quarter rate and caps the PE clock). Rows 3/4 of
    # each 5-block carry the column-norm term split nh+nl+n2.
    ksp = 15 if spat_split else KS
    spat_comb = nc.dram_tensor("spat_comb", [B, ksp, N + RPC], spat_dtype, kind="ExternalInput")
    feat_comb = nc.dram_tensor("feat_comb", [B, KF, N + RPC], feat_dtype, kind="ExternalInput")
    # biases + weights packed partition-major: smalls[p, tensor*2*TILES + b*TILES + t]
    # = value for row t*128+p of batch b. One contiguous 96B-per-partition DMA
    # instead of three 4B-scatter DMAs (which cost ~1 packet per element).
    smalls = nc.dram_tensor("smalls", [128, 3 * B * TILES], FP, kind="ExternalInput")
    out = nc.dram_tensor("out", [B, 128], FP, kind="ExternalOutput")

    with tile.TileContext(nc) as tc:
        with (
            tc.tile_pool(name="const", bufs=1) as cpool,
            tc.tile_pool(name="psum", bufs=2, space="PSUM") as ppool,
            tc.tile_pool(name="ebuf", bufs=2) as epool,
            tc.tile_pool(name="junk", bufs=2) as jpool,
            tc.tile_pool(name="small", bufs=3) as spool,
            tc.tile_pool(name="accs", bufs=1) as apool,
        ):
            # --- load constants ---
            sm = cpool.tile([128, 3 * B * TILES], FP, tag="smalls")
            nc.sync.dma_start(sm[:], smalls[:])
            bx = [sm[:, b * TILES : (b + 1) * TILES] for b in range(B)]
            bf = [sm[:, (B + b) * TILES : (B + b + 1) * TILES] for b in range(B)]
            wt = [sm[:, (2 * B + b) * TILES : (2 * B + b + 1) * TILES] for b in range(B)]
            def load_split(dram, b, shape, dt_, tag):
                # column-chunked DMA: 4 queues in parallel instead of one
                # serialized ~30us transfer
                t_ = cpool.tile(shape, dt_, tag=tag)
                cw = (N + RPC) // 4
                for c in range(4):
                    nc.sync.dma_start(
                        t_[:, c * cw : (c + 1) * cw], dram[b][:, c * cw : (c + 1) * cw]
                    )
                return t_

            scomb, fcomb = [], []
            for b in range(B):
                scomb.append(load_split(spat_comb, b, [ksp, N + RPC], spat_dtype, f"shi{b}"))
                fcomb.append(load_split(feat_comb, b, [KF, N + RPC], feat_dtype, f"fcomb{b}"))

            outsb = apool.tile([128, B], FP, tag="outsb")

            # PE p-state warmup: ~24 dense bf16 matmuls ramp the tensor engine
            # to full clock before the fp32 spatial matmuls start (fp32 at mid
            # p-state costs 2x; see P3 "HAM warmup").
            n_warm = int(os.environ.get("DFL_WARMUP", "24"))
            if n_warm:
                wsrc = cpool.tile([128, 512], mybir.dt.bfloat16, tag="warm")
                nc.gpsimd.memset(wsrc[:], 1.0)
                for k in range(n_warm):
                    pw = ppool.tile([128, 2048], FP, tag="ps")
                    nc.tensor.matmul(
                        pw[:, 0:512], wsrc[:, 0:128], wsrc[:], start=True, stop=True
                    )

            for b in range(B):
                accq = apool.tile([128, TILES], FP, tag=f"accq{b}")
                for t in range(TILES):
                    spart = spool.tile([128, 4], FP, tag="spart")
                    e1 = epool.tile([128, N], FP, tag="e1")
                    e2 = epool.tile([128, N], FP, tag="e2")
                    # spatial & feature score blocks + exp, 2048 cols at a time
                    for half, (comb_t, bias_t, ebuf) in enumerate(
                        (
                            (scomb[b], bx[b], e1),
                            (scomb[b], bx[b], e1),
                            (fcomb[b], bf[b], e2),
                            (fcomb[b], bf[b], e2),
                        )
                    ):
                        half_is = half % 2
                        col0 = half_is * 2048
                        ps = ppool.tile([128, 2048], FP, tag="ps")
                        lhs_sl = slice(N + t * 128, N + (t + 1) * 128)
                        for k in range(4):
                            nc.tensor.matmul(
                                ps[:, k * 512 : (k + 1) * 512],
                                comb_t[:, lhs_sl],
                                comb_t[:, col0 + k * 512 : col0 + (k + 1) * 512],
                                start=True,
                                stop=True,
                            )
                        nc.scalar.activation(
                            ebuf[:, col0 : col0 + 2048],
                            ps[:],
                            AF.Exp,
                            bias=bias_t[:, t : t + 1],
                            accum_out=spart[:, 2 * (half // 2) + half_is : 2 * (half // 2) + half_is + 1],
                        )
                    # row sums s1 (spatial), s2 (feature) and derived scalars
                    sums = spool.tile([128, 2], FP, tag="sums")
                    nc.vector.tensor_reduce(
                        sums[:], spart[:].rearrange("p (m c) -> p m c", c=2), axis=AX.X, op=OP.add
                    )
                    rec = spool.tile([128, 2], FP, tag="rec")
                    nc.vector.reciprocal(rec[:], sums[:])
                    cc = spool.tile([128, 1], FP, tag="cc")
                    nc.vector.tensor_tensor(cc[:], sums[:, 0:1], rec[:, 1:2], op=OP.mult)
                    # d = c*e2 - e1 (in place into e2); then sum d^2 split S/V
                    nc.vector.scalar_tensor_tensor(
                        e2[:], e2[:], cc[:], e1[:], op0=OP.mult, op1=OP.subtract
                    )
                    qa = spool.tile([128, 1], FP, tag="qa")
                    qv = spool.tile([128, 1], FP, tag="qv")
                    junk = jpool.tile([128, ACT_COLS], FP, tag="junk")
                    nc.scalar.activation(
                        junk[:],
                        e2[:, 0:ACT_COLS],
                        AF.Square,
                        scale=rec[:, 0:1],
                        accum_out=qa[:],
                    )
                    nc.vector.scalar_tensor_tensor(
                        out=e2[:, ACT_COLS:],
                        in0=e2[:, ACT_COLS:],
                        scalar=1.0,
                        in1=e2[:, ACT_COLS:],
                        op0=OP.mult,
                        op1=OP.mult,
                        accum_out=qv[:],
                    )
                    # loss rows = qa + inv1^2 * qv
                    i2 = spool.tile([128, 1], FP, tag="i2")
                    nc.vector.tensor_tensor(i2[:], rec[:, 0:1], rec[:, 0:1], op=OP.mult)
                    qv2 = spool.tile([128, 1], FP, tag="qv2")
                    nc.vector.tensor_tensor(qv2[:], qv[:], i2[:], op=OP.mult)
                    nc.vector.tensor_tensor(accq[:, t : t + 1], qa[:], qv2[:], op=OP.add)
                # weighted reduce over this batch's 4 i-tiles
                lw = spool.tile([128, TILES], FP, tag="lw")
                nc.vector.tensor_tensor(lw[:], accq[:], wt[b], op=OP.mult)
                nc.vector.tensor_reduce(outsb[:, b : b + 1], lw[:], axis=AX.X, op=OP.add)

            for b in range(B):
                nc.sync.dma_start(out[b].rearrange("(p o) -> p o", o=1), outsb[:, b : b + 1])

    _fix_walrus_incompat(nc)
    return nc


_NC_CACHE = {}


def _get_nc(use_fp32r=True):
    """Default: fp32 spatial scores (magnitudes ~1e3 make fp32r's relaxed
    precision visible in the loss), fp32r feature scores (magnitudes ~1e2,
    error negligible). DFL_NO_FP32R=1 forces fp32 everywhere;
    DFL_ALL_FP32R=1 forces fp32r everywhere."""
    if os.environ.get("DFL_ALL_FP32R", "") == "1":
        key, dts, split = "rr", (FPR, FPR), False
    elif os.environ.get("DFL_SPAT_FP32", "") == "1":
        key, dts, split = "fr", (FP, FPR), False
    elif not use_fp32r:
        key, dts, split = "ff", (FP, FP), False
    else:
        key, dts, split = "h3", (mybir.dt.float16, FPR), True
    if key not in _NC_CACHE:
        _NC_CACHE[key] = (_build_nc(*dts, spat_split=split), split)
    return _NC_CACHE[key]


def _prep_inputs(points, pointfea1, pointfea2, weights, spat_split=True):
    """Host-side sharding + operand layout. Returns per-core input maps."""
    s2inv = np.float64(1.0) / (SIGMA * SIGMA)
    x = points.astype(np.float64)        # [B, N, 3]
    f1 = pointfea1.astype(np.float64)    # [B, N, D]
    f2 = pointfea2.astype(np.float64)
    w = weights.astype(np.float32)

    xT = np.swapaxes(x, 1, 2)            # [B, 3, N]
    f1T = np.swapaxes(f1, 1, 2)          # [B, D, N]
    f2T = np.swapaxes(f2, 1, 2)

    xn = np.sum(x * x, axis=2)           # [B, N]
    f1n = np.sum(f1 * f1, axis=2)
    f2n = np.sum(f2 * f2, axis=2)

    in_maps = []
    for c in range(NCORES):
        sl = slice(c * RPC, (c + 1) * RPC)
        if spat_split:
            # fp16 hi/lo decomposition, stacked: K=15 = 3 blocks of 5.
            # score = hi.hi + hi.lo + lo.hi -> lhsT blocks [hi;hi;lo],
            # rhs blocks [hi;lo;hi]. Within each 5-block: rows0-2 = coords,
            # rows3/4 = norm pieces (nh+nl via hi/lo row3, n2 via lo-lhs row4).
            y = 2.0 * s2inv * xT
            n = -s2inv * xn
            yh = y.astype(np.float16)
            yl = (y - yh.astype(np.float64)).astype(np.float16)
            nh = n.astype(np.float16)
            nl = (n - nh.astype(np.float64)).astype(np.float16)
            n2 = (n - nh.astype(np.float64) - nl.astype(np.float64)).astype(np.float16)
            xh = xT[:, :, sl].astype(np.float16)
            xl = (xT[:, :, sl] - xh.astype(np.float64)).astype(np.float16)
            hi_r = np.zeros((B, 5, N), np.float16)   # rhs hi block
            lo_r = np.zeros((B, 5, N), np.float16)   # rhs lo block
            hi_l = np.zeros((B, 5, RPC), np.float16)  # lhsT hi block
            lo_l = np.zeros((B, 5, RPC), np.float16)  # lhsT lo block
            hi_r[:, :3] = yh
            hi_r[:, 3] = nh
            hi_r[:, 4] = n2
            lo_r[:, :3] = yl
            lo_r[:, 3] = nl
            hi_l[:, :3] = xh
            hi_l[:, 3] = 1.0
            lo_l[:, :3] = xl
            lo_l[:, 4] = 1.0
            spat_comb = np.empty((B, 15, N + RPC), np.float16)
            spat_comb[:, 0:5, :N] = hi_r
            spat_comb[:, 5:10, :N] = lo_r
            spat_comb[:, 10:15, :N] = hi_r
            spat_comb[:, 0:5, N:] = hi_l
            spat_comb[:, 5:10, N:] = hi_l
            spat_comb[:, 10:15, N:] = lo_l
        else:
            spat_comb = np.empty((B, KS, N + RPC), np.float32)
            spat_comb[:, :3, :N] = 2.0 * s2inv * xT
            spat_comb[:, 3, :N] = -s2inv * xn
            spat_comb[:, :3, N:] = xT[:, :, sl]
            spat_comb[:, 3, N:] = 1.0
        feat_comb = np.empty((B, KF, N + RPC), np.float32)
        feat_comb[:, :D, :N] = 2.0 * f2T
        feat_comb[:, D, :N] = -f2n
        feat_comb[:, :D, N:] = f1T[:, :, sl]
        feat_comb[:, D, N:] = 1.0
        # smalls[p, tensor*B*TILES + b*TILES + t] = value for row t*128+p
        smalls = np.empty((128, 3 * B * TILES), np.float32)
        for b in range(B):
            bx = (-s2inv * xn[b, sl]).astype(np.float32).reshape(TILES, 128)
            bfv = (-f1n[b, sl]).astype(np.float32).reshape(TILES, 128)
            wv = w[b, sl].reshape(TILES, 128)
            smalls[:, b * TILES : (b + 1) * TILES] = bx.T
            smalls[:, (B + b) * TILES : (B + b + 1) * TILES] = bfv.T
            smalls[:, (2 * B + b) * TILES : (2 * B + b + 1) * TILES] = wv.T
        in_maps.append(
            {"spat_comb": spat_comb, "feat_comb": feat_comb, "smalls": smalls}
        )
    return in_maps


def kernel(points, pointfea1, pointfea2, weights):
    global LAST_RESULT
    nc, split = _get_nc(use_fp32r=os.environ.get("DFL_NO_FP32R", "") != "1")
    in_maps = _prep_inputs(points, pointfea1, pointfea2, weights, spat_split=split)
    res = run_bass_kernel_spmd(nc, in_maps, core_ids=list(range(NCORES)))
    LAST_RESULT = res
    total = np.zeros(B, np.float64)
    for m in res.results:
        total += m["out"].astype(np.float64).sum(axis=1)
    return total.astype(np.float32)


# revision 36
# speedup vs baseline: 1.6829x; 1.0013x over previous
"""Trainium2 Bass kernel for nn_DeepFeatureLoss (pairwise softmax-correspondence loss).

Math (per batch b):
    P = softmax_j(-||x_i - x_j||^2),   x = points / SIGMA
    F = softmax_j(-||f1_i - f2_j||^2)
    out[b] = sum_i w_i * sum_j (P_ij - F_ij)^2

Strategy: shard rows i across 8 cores (512 rows each). Host precomputes
transposed/augmented matmul operands so the device kernel is pure compute:
    score_spatial[i,j] = (2/s^2) x_i.x_j - (1/s^2)|x_j|^2   (K=4 matmul, ones row)
    exp with per-row bias -(1/s^2)|x_i|^2  ->  exp(-||xi-xj||^2/s^2)  (<= 1, no overflow)
and similarly for features (K=33). Row sums come free via activation accum.
    sum_j (P-F)^2 = (1/s1^2) * sum_j (c*e2 - e1)^2,  c = s1/s2
computed with one fused scalar_tensor_tensor pass + split square-reduce
(ScalarE on the first 1024 cols, VectorE tensor_tensor_reduce on the rest).
Per-core partial losses [128 lanes, B] are summed on host.
"""

import os
import sys

import numpy as np

sys.path.insert(0, "/opt/trn_rl_repo")

import concourse.bass as bass
import concourse.tile as tile
from concourse import mybir
from concourse.bass_utils import run_bass_kernel_spmd

# If the environment sets BASS_TRACE, run_bass_kernel_spmd imports
# antenv.axon_hooks; provide a null-hook fallback when the image lacks it.
try:
    import antenv.axon_hooks  # noqa: F401
except Exception:
    try:
        import types

        import antenv

        _m = types.ModuleType("antenv.axon_hooks")
        _m._hook = None
        _m.set_axon_ntff_profile_hook = lambda h: setattr(_m, "_hook", h)
        _m.get_axon_ntff_profile_hook = lambda: _m._hook
        sys.modules["antenv.axon_hooks"] = _m
        antenv.axon_hooks = _m
    except Exception:
        pass

SIGMA = 0.05
B = 2
N = 4096
D = 32
NCORES = 8
RPC = N // NCORES          # rows per core = 512
TILES = RPC // 128         # i-tiles per core per batch = 4
KF = D + 1                 # feature contraction with ones row
KS = 4                     # spatial contraction (3 coords + ones row)
ACT_COLS = 768             # columns of the square-reduce done on ScalarE

FP = mybir.dt.float32
FPR = mybir.dt.float32r    # 4x faster PE streaming, fp32 data
AX = mybir.AxisListType
OP = mybir.AluOpType
AF = mybir.ActivationFunctionType

LAST_RESULT = None         # test harness introspection


def _fix_walrus_incompat(nc):
    """This container's walrus codegen fits exactly ONE sync-wait per engine
    instruction struct (Tile's scheduler freely emits several) and rejects the
    EVENT_SEMAPHORE_RANGE_CLEAR raw-ISA instruction Tile emits at context
    exit. Rewrite: (a) every multi-wait instruction becomes (n-1) same-engine
    EventSemaphore waits followed by the instruction with the final wait;
    (b) the range-clear becomes one sem-wr-imm(0) EventSemaphore per sem."""
    import re

    from bass_rust import SyncInfo, SyncUpdate

    fn = nc.m.functions[0]
    originals = [(blk, list(blk.instructions)) for blk in fn.blocks]
    rebuilt = []
    for blk, insts in originals:
        out = []
        for inst in insts:
            tname = type(inst).__name__
            si = inst.sync_info
            if tname == "InstISA" and "EVENT_SEMAPHORE_RANGE_CLEAR" in inst.concise():
                m = re.search(r"range_first=(\d+) range_last=(\d+)", inst.concise())
                first, last = int(m.group(1)), int(m.group(2))
                for sem in range(first, last + 1):
                    ev = mybir.InstEventSemaphore(
                        name=nc.get_next_instruction_name(),
                        engine=inst.engine,
                        sync_info=SyncInfo(
                            on_wait=list(si.on_wait) if si and sem == first else [],
                            on_update=[
                                SyncUpdate(
                                    sync_type="semaphore",
                                    id=sem,
                                    ant_name=f"semclear_{sem}",
                                    update_mode="sem-wr-imm",
                                    update_value=0,
                                    update_reg=None,
                                )
                            ],
                        ),
                    )
                    nc.register_instruction(ev, overwrite=True)
                    out.append(ev)
                continue
            if si is not None and len(si.on_wait) > 1:
                waits = list(si.on_wait)
                for w in waits[:-1]:
                    ev = mybir.InstEventSemaphore(
                        name=nc.get_next_instruction_name(),
                        engine=inst.engine,
                        sync_info=SyncInfo(on_wait=[w], on_update=[]),
                    )
                    nc.register_instruction(ev, overwrite=True)
                    out.append(ev)
                inst.sync_info = SyncInfo(
                    on_wait=[waits[-1]], on_update=list(si.on_update)
                )
            out.append(inst)
        rebuilt.append((blk, out))
    for blk, out in rebuilt:
        blk.instructions[:] = out


def _build_nc(spat_dtype, feat_dtype, spat_split=False):
    nc = bass.Bass()

    # rhs (cols 0:N) and the local lhsT block (cols N:N+RPC) share one DRAM
    # tensor so each matmul family depends on exactly ONE input DMA — walrus's
    # core_v3 LDWEIGHTS struct only fits a single sync-wait.
    # spat_split: fp16 hi/lo decomposition of the spatial operands; the three
    # products hi.hi + hi.lo + lo.hi (22-bit effective mantissa) are STACKED
    # along the contraction axis — lhsT rows [hi;hi;lo], rhs rows [hi;lo;hi],
    # K=15 — so the whole compensated product is ONE full-rate fp16 matmul
    # (fp32 matmul runs at quarter rate and caps the PE clock). Rows 3/4 of
    # each 5-block carry the column-norm term split nh+nl+n2.
    ksp = 15 if spat_split else KS
    spat_comb = nc.dram_tensor("spat_comb", [B, ksp, N + RPC], spat_dtype, kind="ExternalInput")
    feat_comb = nc.dram_tensor("feat_comb", [B, KF, N + RPC], feat_dtype, kind="ExternalInput")
    # biases + weights packed partition-major: smalls[p, tensor*2*TILES + b*TILES + t]
    # = value for row t*128+p of batch b. One contiguous 96B-per-partition DMA
    # instead of three 4B-scatter DMAs (which cost ~1 packet per element).
    smalls = nc.dram_tensor("smalls", [128, 3 * B * TILES], FP, kind="ExternalInput")
    out = nc.dram_tensor("out", [B, 128], FP, kind="ExternalOutput")

    with tile.TileContext(nc) as tc:
        with (
            tc.tile_pool(name="const", bufs=1) as cpool,
            tc.tile_pool(name="psum", bufs=2, space="PSUM") as ppool,
            tc.tile_pool(name="ebuf", bufs=3) as epool,
            tc.tile_pool(name="junk", bufs=3) as jpool,
            tc.tile_pool(name="small", bufs=6) as spool,
            tc.tile_pool(name="accs", bufs=1) as apool,
        ):
            # --- load constants ---
            sm = cpool.tile([128, 3 * B * TILES], FP, tag="smalls")
            nc.sync.dma_start(sm[:], smalls[:])
            bx = [sm[:, b * TILES : (b + 1) * TILES] for b in range(B)]
            bf = [sm[:, (B + b) * TILES : (B + b + 1) * TILES] for b in range(B)]
            wt = [sm[:, (2 * B + b) * TILES : (2 * B + b + 1) * TILES] for b in range(B)]
            def load_split(dram, b, shape, dt_, tag):
                # column-chunked DMA: 4 queues in parallel instead of one
                # serialized ~30us transfer
                t_ = cpool.tile(shape, dt_, tag=tag)
                cw = (N + RPC) // 4
                for c in range(4):
                    nc.sync.dma_start(
                        t_[:, c * cw : (c + 1) * cw], dram[b][:, c * cw : (c + 1) * cw]
                    )
                return t_

            scomb, fcomb = [], []
            for b in range(B):
                scomb.append(load_split(spat_comb, b, [ksp, N + RPC], spat_dtype, f"shi{b}"))
                fcomb.append(load_split(feat_comb, b, [KF, N + RPC], feat_dtype, f"fcomb{b}"))

            outsb = apool.tile([128, B], FP, tag="outsb")

            # PE p-state warmup: ~24 dense bf16 matmuls ramp the tensor engine
            # to full clock before the fp32 spatial matmuls start (fp32 at mid
            # p-state costs 2x; see P3 "HAM warmup").
            n_warm = int(os.environ.get("DFL_WARMUP", "24"))
            if n_warm:
                wsrc = cpool.tile([128, 512], mybir.dt.bfloat16, tag="warm")
                nc.gpsimd.memset(wsrc[:], 1.0)
                for k in range(n_warm):
                    pw = ppool.tile([128, 2048], FP, tag="ps")
                    nc.tensor.matmul(
                        pw[:, 0:512], wsrc[:, 0:128], wsrc[:], start=True, stop=True
                    )

            for b in range(B):
                accq = apool.tile([128, TILES], FP, tag=f"accq{b}")
                for t in range(TILES):
                    spart = spool.tile([128, 4], FP, tag="spart")
                    e1 = epool.tile([128, N], FP, tag="e1")
                    e2 = epool.tile([128, N], FP, tag="e2")
                    # spatial & feature score blocks + exp, 2048 cols at a time
                    for half, (comb_t, bias_t, ebuf) in enumerate(
                        (
                            (scomb[b], bx[b], e1),
                            (scomb[b], bx[b], e1),
                            (fcomb[b], bf[b], e2),
                            (fcomb[b], bf[b], e2),
                        )
                    ):
                        half_is = half % 2
                        col0 = half_is * 2048
                        ps = ppool.tile([128, 2048], FP, tag="ps")
                        lhs_sl = slice(N + t * 128, N + (t + 1) * 128)
                        for k in range(4):
                            nc.tensor.matmul(
                                ps[:, k * 512 : (k + 1) * 512],
                                comb_t[:, lhs_sl],
                                comb_t[:, col0 + k * 512 : col0 + (k + 1) * 512],
                                start=True,
                                stop=True,
                            )
                        nc.scalar.activation(
                            ebuf[:, col0 : col0 + 2048],
                            ps[:],
                            AF.Exp,
                            bias=bias_t[:, t : t + 1],
                            accum_out=spart[:, 2 * (half // 2) + half_is : 2 * (half // 2) + half_is + 1],
                        )
                    # row sums s1 (spatial), s2 (feature) and derived scalars
                    sums = spool.tile([128, 2], FP, tag="sums")
                    nc.vector.tensor_reduce(
                        sums[:], spart[:].rearrange("p (m c) -> p m c", c=2), axis=AX.X, op=OP.add
                    )
                    rec = spool.tile([128, 2], FP, tag="rec")
                    nc.vector.reciprocal(rec[:], sums[:])
                    cc = spool.tile([128, 1], FP, tag="cc")
                    nc.vector.tensor_tensor(cc[:], sums[:, 0:1], rec[:, 1:2], op=OP.mult)
                    # d = c*e2 - e1 (in place into e2); then sum d^2 split S/V
                    nc.vector.scalar_tensor_tensor(
                        e2[:], e2[:], cc[:], e1[:], op0=OP.mult, op1=OP.subtract
                    )
                    qa = spool.tile([128, 1], FP, tag="qa")
                    qv = spool.tile([128, 1], FP, tag="qv")
                    junk = jpool.tile([128, ACT_COLS], FP, tag="junk")
                    nc.scalar.activation(
                        junk[:],
                        e2[:, 0:ACT_COLS],
                        AF.Square,
                        scale=rec[:, 0:1],
                        accum_out=qa[:],
                    )
                    nc.vector.scalar_tensor_tensor(
                        out=e2[:, ACT_COLS:],
                        in0=e2[:, ACT_COLS:],
                        scalar=1.0,
                        in1=e2[:, ACT_COLS:],
                        op0=OP.mult,
                        op1=OP.mult,
                        accum_out=qv[:],
                    )
                    # loss rows = qa + inv1^2 * qv
                    i2 = spool.tile([128, 1], FP, tag="i2")
                    nc.vector.tensor_tensor(i2[:], rec[:, 0:1], rec[:, 0:1], op=OP.mult)
                    qv2 = spool.tile([128, 1], FP, tag="qv2")
                    nc.vector.tensor_tensor(qv2[:], qv[:], i2[:], op=OP.mult)
                    nc.vector.tensor_tensor(accq[:, t : t + 1], qa[:], qv2[:], op=OP.add)
                # weighted reduce over this batch's 4 i-tiles
                lw = spool.tile([128, TILES], FP, tag="lw")
                nc.vector.tensor_tensor(lw[:], accq[:], wt[b], op=OP.mult)
                nc.vector.tensor_reduce(outsb[:, b : b + 1], lw[:], axis=AX.X, op=OP.add)

            for b in range(B):
                nc.sync.dma_start(out[b].rearrange("(p o) -> p o", o=1), outsb[:, b : b + 1])

    _fix_walrus_incompat(nc)
    return nc


_NC_CACHE = {}


def _get_nc(use_fp32r=True):
    """Default: fp32 spatial scores (magnitudes ~1e3 make fp32r's relaxed
    precision visible in the loss), fp32r feature scores (magnitudes ~1e2,
    error negligible). DFL_NO_FP32R=1 forces fp32 everywhere;
    DFL_ALL_FP32R=1 forces fp32r everywhere."""
    if os.environ.get("DFL_ALL_FP32R", "") == "1":
        key, dts, split = "rr", (FPR, FPR), False
    elif os.environ.get("DFL_SPAT_FP32", "") == "1":
        key, dts, split = "fr", (FP, FPR), False
    elif not use_fp32r:
        key, dts, split = "ff", (FP, FP), False
    else:
        key, dts, split = "h3", (mybir.dt.float16, FPR), True
    if key not in _NC_CACHE:
        _NC_CACHE[key] = (_build_nc(*dts, spat_split=split), split)
    return _NC_CACHE[key]


def _prep_inputs(points, pointfea1, pointfea2, weights, spat_split=True):
    """Host-side sharding + operand layout. Returns per-core input maps."""
    s2inv = np.float64(1.0) / (SIGMA * SIGMA)
    x = points.astype(np.float64)        # [B, N, 3]
    f1 = pointfea1.astype(np.float64)    # [B, N, D]
    f2 = pointfea2.astype(np.float64)
    w = weights.astype(np.float32)

    xT = np.swapaxes(x, 1, 2)            # [B, 3, N]
    f1T = np.swapaxes(f1, 1, 2)          # [B, D, N]
    f2T = np.swapaxes(f2, 1, 2)

    xn = np.sum(x * x, axis=2)           # [B, N]
    f1n = np.sum(f1 * f1, axis=2)
    f2n = np.sum(f2 * f2, axis=2)

    in_maps = []
    for c in range(NCORES):
        sl = slice(c * RPC, (c + 1) * RPC)
        if spat_split:
            # fp16 hi/lo decomposition, stacked: K=15 = 3 blocks of 5.
            # score = hi.hi + hi.lo + lo.hi -> lhsT blocks [hi;hi;lo],
            # rhs blocks [hi;lo;hi]. Within each 5-block: rows0-2 = coords,
            # rows3/4 = norm pieces (nh+nl via hi/lo row3, n2 via lo-lhs row4).
            y = 2.0 * s2inv * xT
            n = -s2inv * xn
            yh = y.astype(np.float16)
            yl = (y - yh.astype(np.float64)).astype(np.float16)
            nh = n.astype(np.float16)
            nl = (n - nh.astype(np.float64)).astype(np.float16)
            n2 = (n - nh.astype(np.float64) - nl.astype(np.float64)).astype(np.float16)
            xh = xT[:, :, sl].astype(np.float16)
            xl = (xT[:, :, sl] - xh.astype(np.float64)).astype(np.float16)
            hi_r = np.zeros((B, 5, N), np.float16)   # rhs hi block
            lo_r = np.zeros((B, 5, N), np.float16)   # rhs lo block
            hi_l = np.zeros((B, 5, RPC), np.float16)  # lhsT hi block
            lo_l = np.zeros((B, 5, RPC), np.float16)  # lhsT lo block
            hi_r[:, :3] = yh
            hi_r[:, 3] = nh
            hi_r[:, 4] = n2
            lo_r[:, :3] = yl
            lo_r[:, 3] = nl
            hi_l[:, :3] = xh
            hi_l[:, 3] = 1.0
            lo_l[:, :3] = xl
            lo_l[:, 4] = 1.0
            spat_comb = np.empty((B, 15, N + RPC), np.float16)
            spat_comb[:, 0:5, :N] = hi_r
            spat_comb[:, 5:10, :N] = lo_r
            spat_comb[:, 10:15, :N] = hi_r
            spat_comb[:, 0:5, N:] = hi_l
            spat_comb[:, 5:10, N:] = hi_l
            spat_comb[:, 10:15, N:] = lo_l
        else:
            spat_comb = np.empty((B, KS, N + RPC), np.float32)
            spat_comb[:, :3, :N] = 2.0 * s2inv * xT
            spat_comb[:, 3, :N] = -s2inv * xn
            spat_comb[:, :3, N:] = xT[:, :, sl]
            spat_comb[:, 3, N:] = 1.0
        feat_comb = np.empty((B, KF, N + RPC), np.float32)
        feat_comb[:, :D, :N] = 2.0 * f2T
        feat_comb[:, D, :N] = -f2n
        feat_comb[:, :D, N:] = f1T[:, :, sl]
        feat_comb[:, D, N:] = 1.0
        # smalls[p, tensor*B*TILES + b*TILES + t] = value for row t*128+p
        smalls = np.empty((128, 3 * B * TILES), np.float32)
        for b in range(B):
            bx = (-s2inv * xn[b, sl]).astype(np.float32).reshape(TILES, 128)
            bfv = (-f1n[b, sl]).astype(np.float32).reshape(TILES, 128)
            wv = w[b, sl].reshape(TILES, 128)
            smalls[:, b * TILES : (b + 1) * TILES] = bx.T
            smalls[:, (B + b) * TILES : (B + b + 1) * TILES] = bfv.T
            smalls[:, (2 * B + b) * TILES : (2 * B + b + 1) * TILES] = wv.T
        in_maps.append(
            {"spat_comb": spat_comb, "feat_comb": feat_comb, "smalls": smalls}
        )
    return in_maps


def kernel(points, pointfea1, pointfea2, weights):
    global LAST_RESULT
    nc, split = _get_nc(use_fp32r=os.environ.get("DFL_NO_FP32R", "") != "1")
    in_maps = _prep_inputs(points, pointfea1, pointfea2, weights, spat_split=split)
    res = run_bass_kernel_spmd(nc, in_maps, core_ids=list(range(NCORES)))
    LAST_RESULT = res
    total = np.zeros(B, np.float64)
    for m in res.results:
        total += m["out"].astype(np.float64).sum(axis=1)
    return total.astype(np.float32)
